# revision 3
# baseline (speedup 1.0000x reference)
"""Bahdanau-attention kernel for Trainium2 (8 NeuronCores).

Mathematical note: the reference computes
    score  = tanh(q@Ws + keys@Wh) @ W          # [B, T, 1]
    attend = softmax(score, axis=-1)           # softmax over a size-1 axis
    out    = sum(keys * attend, axis=1)
A softmax over a single-element axis is identically 1.0 (exp(x-x)/exp(x-x)),
bit-exactly in fp32, so the output is exactly keys.sum(axis=1).  The kernel
therefore only needs to reduce keys [32, 4096, 512] over T — a pure
memory-bound reduction (256 MB of reads).

Strategy: data-parallel over batch B=32 across 8 cores (4 batches/core,
32 MB/core).  Per core, each batch [4096, 512] is streamed through SBUF in
[128, 2048] tiles (1 MB contiguous DMAs; partition p holds 4 consecutive
t-rows), accumulated pairwise on the vector engine (fp32 tensor_tensor,
N=1024), folded to [128, 512], and the final cross-partition reduction is
one matmul with a ones-vector on the tensor engine.  Predicted bottleneck:
HBM DMA at ~358 GB/s/core → ~94 us.
"""

import numpy as np

N_CORES = 8
B, T, D = 32, 4096, 512
BPC = B // N_CORES          # batches per core = 4

TILE_T = 512                # t-rows per in-tile
ROWS_PER_PART = TILE_T // 128   # = 4 consecutive t-rows per partition
TILE_F = ROWS_PER_PART * D      # = 2048 free elements per partition (8 KB)
TILES_PER_BATCH = T // TILE_T   # = 8
ACC_W = 2 * D               # accumulator width 1024 (2 d-groups)
IN_BUFS = 8

_CACHE = {}


def _build_nc():
    import concourse.bacc as bacc
    import concourse.bass as bass
    import concourse.mybir as mybir
    import concourse.tile as tile

    nc = bacc.Bacc(
        "TRN2",
        target_bir_lowering=False,
        debug=False,
        num_devices=N_CORES,
    )
    keys = nc.dram_tensor(
        "keys", [BPC, T, D], mybir.dt.float32, kind="ExternalInput"
    ).ap()
    out = nc.dram_tensor(
        "out", [BPC, D], mybir.dt.float32, kind="ExternalOutput"
    ).ap()

    # keys[b] viewed as tiles: row r = i*TILE_T + p*ROWS_PER_PART + n
    kv = keys.rearrange(
        "b (i p n) d -> b i p (n d)", p=128, n=ROWS_PER_PART
    )  # [BPC, TILES_PER_BATCH, 128, TILE_F]

    f32 = mybir.dt.float32
    with tile.TileContext(nc) as tc:
        with (
            tc.tile_pool(name="ones", bufs=1) as ones_pool,
            tc.tile_pool(name="inp", bufs=IN_BUFS) as in_pool,
            tc.tile_pool(name="acc", bufs=2) as acc_pool,
            tc.tile_pool(name="psum", bufs=2, space="PSUM") as psum_pool,
            tc.tile_pool(name="stage", bufs=2) as stage_pool,
        ):
            ones_t = ones_pool.tile([128, 1], f32)
            nc.gpsimd.memset(ones_t[:], 1.0)

            for b in range(BPC):
                acc = acc_pool.tile([128, ACC_W], f32)
                for i in range(TILES_PER_BATCH):
                    t = in_pool.tile([128, TILE_F], f32, tag="inp")
                    nc.sync.dma_start(t[:], kv[b, i])
                    if i == 0:
                        # initializes acc, no memset needed
                        nc.vector.tensor_add(
                            acc[:], t[:, 0:ACC_W], t[:, ACC_W:TILE_F]
                        )
                    else:
                        nc.vector.tensor_add(
                            acc[:], acc[:], t[:, 0:ACC_W]
                        )
                        nc.vector.tensor_add(
                            acc[:], acc[:], t[:, ACC_W:TILE_F]
                        )
                # fold 1024 -> 512
                nc.vector.tensor_add(acc[:, 0:D], acc[:, 0:D], acc[:, D:ACC_W])
                # cross-partition reduce: [1,512] = ones.T @ acc[:, :512]
                psum_t = psum_pool.tile([1, D], f32)
                nc.tensor.matmul(
                    psum_t[:], ones_t[:], acc[:, 0:D], start=True, stop=True
                )
                stage = stage_pool.tile([1, D], f32)
                nc.vector.tensor_copy(stage[:], psum_t[:])
                nc.sync.dma_start(out[b : b + 1, :], stage[:])
    nc.compile()
    return nc


def _get_nc():
    if "nc" not in _CACHE:
        _CACHE["nc"] = _build_nc()
    return _CACHE["nc"]


def _run(keys_full, trace=False):
    from concourse.bass_utils import run_bass_kernel_spmd

    nc = _get_nc()
    keys_np = np.ascontiguousarray(np.asarray(keys_full, dtype=np.float32))
    in_maps = [
        {"keys": keys_np[c * BPC : (c + 1) * BPC]} for c in range(N_CORES)
    ]
    res = run_bass_kernel_spmd(nc, in_maps, list(range(N_CORES)), trace=trace)
    out = np.concatenate(
        [res.results[c]["out"] for c in range(N_CORES)], axis=0
    )
    return out, res


def kernel(query, keys, Ws, Wh, W):
    # softmax over the size-1 score axis is exactly 1.0, so the output is
    # keys.sum(axis=1); query/Ws/Wh/W do not affect the result.
    out, _ = _run(keys, trace=False)
    return out


# revision 14
# speedup vs baseline: 1.1890x; 1.1890x over previous
"""Bahdanau-attention kernel for Trainium2 (8 NeuronCores).

Mathematical note: the reference computes
    score  = tanh(q@Ws + keys@Wh) @ W          # [B, T, 1]
    attend = softmax(score, axis=-1)           # softmax over a size-1 axis
    out    = sum(keys * attend, axis=1)
A softmax over a single-element axis is identically 1.0 (exp(x-x) == 1,
sum == 1, bit-exact in fp32), so the output is exactly keys.sum(axis=1).
The kernel therefore only needs to reduce keys [32, 4096, 512] over T — a
pure memory-bound reduction (256 MB of reads).

Strategy: data-parallel over batch B=32 across 8 cores (4 batches/core,
32 MB/core).  Per core, each batch [4096, 512] is streamed through SBUF in
[128, 2048] tiles (1 MB DMAs, 8 KB contiguous per partition), accumulated
on the vector engine (fp32 tensor_tensor adds, ~80 us busy), folded to
[128, 512], and the final cross-partition reduction is one matmul with a
ones-vector on the tensor engine into PSUM.  Bottleneck: HBM DMA at
~358 GB/s/core → ~94 us stream; measured ~106-110 us end to end
(framework start barrier + engine-table loads ~4.5 us, reduce tail +
drain ~5 us, HBM contention between core pairs accounts for the rest).
slim_sync removes the Bass entry barrier (orders only never-read const
memsets) and the second of two trailing all-engine barriers: -2.3 us.
"""

import numpy as np

N_CORES = 8
B, T, D = 32, 4096, 512
BPC = B // N_CORES          # batches per core = 4

_CACHE = {}


def _build_nc(
    tile_t=512, acc_w=1024, in_bufs=12, rings=1, slim_sync=True, final="pe"
):
    import concourse.bacc as bacc
    import concourse.bass as bass
    import concourse.bass_isa as bass_isa
    import concourse.mybir as mybir
    import concourse.tile as tile

    rows_per_part = tile_t // 128        # consecutive t-rows per partition
    tile_f = rows_per_part * D           # free elements per partition
    tiles_per_batch = T // tile_t
    assert tile_f % acc_w == 0 and acc_w % D == 0

    if slim_sync:
        # Skip the Bass.__init__ entry all-engine barrier (it only orders the
        # framework const memsets, which this kernel never reads — our DMAs
        # can start immediately instead of absorbing engine-start skew).
        orig_barrier = bass.Bass.all_engine_barrier
        bass.Bass.all_engine_barrier = lambda self, *, sem_only=False: None
    try:
        nc = bacc.Bacc(
            "TRN2",
            target_bir_lowering=False,
            debug=False,
            num_devices=N_CORES,
        )
    finally:
        if slim_sync:
            bass.Bass.all_engine_barrier = orig_barrier
    keys = nc.dram_tensor(
        "keys", [BPC, T, D], mybir.dt.float32, kind="ExternalInput"
    ).ap()
    out = nc.dram_tensor(
        "out", [BPC, D], mybir.dt.float32, kind="ExternalOutput"
    ).ap()

    # keys[b] tiled: row r = i*tile_t + p*rows_per_part + n
    kv = keys.rearrange(
        "b (i p n) d -> b i p (n d)", p=128, n=rows_per_part
    )  # [BPC, tiles_per_batch, 128, tile_f]

    f32 = mybir.dt.float32
    tc_ctx = tile.TileContext(nc)
    if slim_sync:
        import types as _types

        from concourse.vector_clock import ScopedClock

        def _slim_drain_and_barrier(self, tick_clock, wait_clock):
            # Same as TileContext._drain_and_barrier but with a single
            # all-engine barrier: the drain already waits on every proc's
            # final tick, and re-execution safety only needs the sem clears
            # ordered before each engine's next-run entry (engines halt at
            # program end, which is a stronger fence than the 2nd barrier).
            drain_inst = self.nc.sync.drain()
            wait_clock.add_sem_waits(
                drain_inst.ins, ScopedClock({None: tick_clock.global_clock})
            )
            self.nc.multi_engine_barrier(list(self.nc.engines))
            popped = self.nc._tile_sem_poison_stack.pop()
            assert popped is self._sem_poison
            self.nc.clear_and_free_semaphores(
                list(self.sems.allocated().values())
            )

        tc_ctx._drain_and_barrier = _types.MethodType(
            _slim_drain_and_barrier, tc_ctx
        )
    with tc_ctx as tc:
        with (
            tc.tile_pool(name="ones", bufs=1) as ones_pool,
            tc.tile_pool(name="inp", bufs=in_bufs) as in_pool,
            tc.tile_pool(name="acc", bufs=2) as acc_pool,
            tc.tile_pool(name="psum", bufs=2, space="PSUM") as psum_pool,
            tc.tile_pool(name="stage", bufs=2) as stage_pool,
        ):
            ones_t = None
            if final == "pe":
                ones_t = ones_pool.tile([128, 1], f32)
                nc.gpsimd.memset(ones_t[:], 1.0)

            for b in range(BPC):
                acc = acc_pool.tile([128, acc_w], f32)
                pending = []  # slices before acc is initialized
                acc_init = False
                for i in range(tiles_per_batch):
                    t = in_pool.tile([128, tile_f], f32, tag="inp")
                    eng = nc.sync if (rings == 1 or i % 2 == 0) else nc.scalar
                    eng.dma_start(t[:], kv[b, i])
                    for j in range(tile_f // acc_w):
                        sl = t[:, j * acc_w : (j + 1) * acc_w]
                        if not acc_init:
                            pending.append(sl)
                            if len(pending) == 2:
                                # acc = s0 + s1 initializes acc, no memset
                                nc.vector.tensor_add(
                                    acc[:], pending[0][:], pending[1][:]
                                )
                                acc_init = True
                        else:
                            nc.vector.tensor_add(acc[:], acc[:], sl[:])
                # fold acc_w -> D
                w = acc_w
                while w > D:
                    h = w // 2
                    nc.vector.tensor_add(acc[:, 0:h], acc[:, 0:h], acc[:, h:w])
                    w = h
                # cross-partition reduce [128,512] -> per-d sums
                if final == "pe":
                    psum_t = psum_pool.tile([1, D], f32)
                    nc.tensor.matmul(
                        psum_t[:], ones_t[:], acc[:, 0:D], start=True, stop=True
                    )
                    stage = stage_pool.tile([1, D], f32)
                    nc.vector.tensor_copy(stage[:], psum_t[:])
                    nc.sync.dma_start(out[b : b + 1, :], stage[:])
                else:  # gpsimd partition_all_reduce, no PE/PSUM needed
                    stage = stage_pool.tile([128, D], f32)
                    nc.gpsimd.partition_all_reduce(
                        stage[:], acc[:, 0:D], 128, bass_isa.ReduceOp.add
                    )
                    nc.sync.dma_start(out[b : b + 1, :], stage[0:1, :])
    nc.compile()
    return nc


def _get_nc(**kw):
    key = tuple(sorted(kw.items()))
    if key not in _CACHE:
        _CACHE[key] = _build_nc(**kw)
    return _CACHE[key]


def _run(keys_full, trace=False, **kw):
    from concourse.bass_utils import run_bass_kernel_spmd

    nc = _get_nc(**kw)
    keys_np = np.ascontiguousarray(np.asarray(keys_full, dtype=np.float32))
    in_maps = [
        {"keys": keys_np[c * BPC : (c + 1) * BPC]} for c in range(N_CORES)
    ]
    res = run_bass_kernel_spmd(nc, in_maps, list(range(N_CORES)), trace=trace)
    out = np.concatenate(
        [res.results[c]["out"] for c in range(N_CORES)], axis=0
    )
    return out, res


def kernel(query, keys, Ws, Wh, W):
    # softmax over the size-1 score axis is exactly 1.0, so the output is
    # keys.sum(axis=1); query/Ws/Wh/W do not affect the result.
    out, _ = _run(keys, trace=False)
    return out
